# revision 2
# baseline (speedup 1.0000x reference)
"""Trainium2 Bass kernel for CausalGraphNetwork (restructured v2).

Computes, for x = step_sequence [B=2, N=512, H=256]:
    h  = relu(x @ W_gc1.T + b_gc1)
    f  = relu(h @ W_gc2.T + b_gc2)
    a  = f @ Wa.T + b_ep1    (Wa = W_ep1[:, :H])
    c  = f @ Wb.T            (Wb = W_ep1[:, H:])
    e[b,i,j,:] = relu(a[b,i,:] + c[b,j,:])
    scores = sigmoid(e @ w_ep2 + b_ep2) * strict_lower_mask

Changes vs v1 baseline:
  - Input DMAs merged into 4 transfers ordered by first use (xt+w1+consts
    first) with f32/bf16 consts bit-packed into the fp8 blob.
  - Scales folded into host packing so h/c/a epilogues are pure
    tensor_scalar ops (any of DVE/ACT/Pool); only the f epilogue needs
    ACT's native scale.  h8 stores 16h, f8 stores 16f; ps3 = 128*(f*W).
  - Critical chain xt -> h(j-tok) -> f(j-tok) -> c -> ct isolated; the
    own-token h1/f1/a chain runs in the shadow (a-path in bf16).
  - Pool engine added as a third e-gen engine; schedule walk rebalanced
    with trace-calibrated costs.
  - Pairwise groups processed in descending order so the last drain is
    the smallest (short tail).
  - PE kept continuously busy through the prologue (pstate stays high).
"""

import ml_dtypes
import numpy as np

import bass_rust
import concourse.bass as bass
import concourse.mybir as mybir
import concourse.tile as tile
from concourse.bass_utils import run_bass_kernel_spmd
from concourse.vector_clock import ScopedClock

B, N, H = 2, 512, 256
NCORES = 8
R = 128  # rows per core
NT = N + R  # 640 token columns: 512 shared j-tokens + 128 own i-tokens
F32 = mybir.dt.float32
BF16 = mybir.dt.bfloat16
FP8 = mybir.dt.float8e4
AF = mybir.ActivationFunctionType
ALU = mybir.AluOpType
DR = mybir.MatmulPerfMode.DoubleRow

E_SCALE = 8.0              # scale carried by ct/at (fp8 e floor)
W2_SCALE = 64.0            # w_ep2 pre-scale (fp8 denormal floor)
SIG_SCALE = 1.0 / (E_SCALE * W2_SCALE)

# blob1 layout (fp8 bytes): xt | w1m | cs32(f32 x 10) | wep2(bf16 x 2)
B1_XT = 0
B1_W1 = 1280
B1_CS = B1_W1 + 512
B1_WEP2 = B1_CS + 40
B1_LEN = B1_WEP2 + 4
# cs32 f32 columns
CS_B1 = 0    # 16*b_gc1           [2]
CS_B2 = 2    # 16*b_gc2           [2]
CS_BA = 4    # 8*b_ep1            [2]
CS_BEP2 = 6  # b_ep2              [1]


def jbx(k: int) -> int:
    """Per-row causal compute extent (multiple of 8, >= 4k+4)."""
    return min(N, ((4 * k + 4) + 7) // 8 * 8)


# ---------------------------------------------------------------------------
# schedule: descending group pairs, fp8-group quota + 3-engine balanced walk
# ---------------------------------------------------------------------------

import json as _json
import os as _os

# schedule-model constants, overridable for sweeps: K2_CFG='{"act_egen_b": 185}'
_CFG = {
    "act_egen_a": 0.833, "act_egen_b": 185.0,
    "pool_egen_a": 0.835, "pool_egen_b": 5.0,
    "dve_b16_a": 0.263, "dve_b16_b": 68.0,
    "dve_f8_a": 0.493, "dve_f8_b": 42.0,
    "sig_a": 1.666, "sig_b": 185.0,
}
_CFG.update(_json.loads(_os.environ.get("K2_CFG", "{}")))


def _cost_egen(eng, mode, jb):
    if eng == "dve":
        if mode == "bf16":
            return _CFG["dve_b16_a"] * jb + _CFG["dve_b16_b"]
        return _CFG["dve_f8_a"] * jb + _CFG["dve_f8_b"]
    if eng == "act":
        return _CFG["act_egen_a"] * jb + _CFG["act_egen_b"]
    return _CFG["pool_egen_a"] * jb + _CFG["pool_egen_b"]  # pool


def _cost_epi(eng, n):
    """Upstream epilogue (tensor_scalar from psum) cost on each engine."""
    if eng == "dve":
        return n * 1.0417 + 125.0
    if eng == "act":
        return n * 0.833 + 150.0
    return n * 0.836 + 5.0


def _sig_cost(jbb):
    return _CFG["sig_a"] * jbb + _CFG["sig_b"]


# upstream stages as walk jobs: (stage, [epi sizes], fixed_engine_or_None)
UP_STAGES = (
    ("xt", (), None),
    ("h0", (512, 512), None),
    ("h1", (128, 128), None),
    ("f0", (512, 512), "act"),   # needs ACT scale
    ("f1", (128, 128), None),    # bf16 out, add+max
    ("c", (512, 512), None),
    ("a", (128, 128), None),
)
# fraction of pairwise work after which each stage is triggered
UP_FRAC = {"xt": 0.04, "h0": 0.18, "h1": 0.30, "f0": 0.44, "f1": 0.56,
           "c": 0.70, "a": 0.80}


def _mk_schedule():
    """Choose fp8/bf16 mode per group and an engine for every e-gen chunk
    and upstream epilogue, minimizing the max engine makespan."""
    # zipper pair order (big, small, big, small, ... , smallest last) so
    # sigmoid+DMA drains spread out instead of stacking at the end
    if _os.environ.get("K2_ORDER", "zipper") == "asc":
        pairs = list(range(16))
        order = []
        for p in pairs:
            order += [2 * p, 2 * p + 1]
    else:
        pairs = []
        for i in range(8):
            pairs += [15 - i, 7 - i]
        order = []
        for p in pairs:
            order += [2 * p + 1, 2 * p]
    # cumulative work fraction at each iteration (for trigger placement)
    w = [sum(jbx(k) for k in range(4 * G, 4 * G + 4)) for G in order]
    tot = float(sum(w))
    cum, acc = [], 0.0
    for x in w:
        cum.append(acc / tot)
        acc += x
    triggers = {}  # iteration index -> list of stage names
    for st, _, _ in UP_STAGES:
        fr = UP_FRAC[st]
        it = next((i for i, cv in enumerate(cum) if cv >= fr), len(order) - 1)
        triggers.setdefault(it, []).append(st)

    def run_walk(modes, up_all_act=False):
        busy = {"dve": 0.0, "act": 0.0, "pool": 0.0}
        pe = 0.96e3  # upstream matmuls per rep (ns)
        assign = {}  # (G, k, c) -> engine
        up_assign = {}  # (stage, idx) -> engine
        seen = set()
        for it, G in enumerate(order):
            for st in triggers.get(it, []):
                sizes = dict((s, (e, f)) for s, e, f in UP_STAGES)[st]
                epis, fixed = sizes
                if up_all_act and not fixed:
                    fixed = "act"
                for i, n in enumerate(epis):
                    if fixed:
                        busy[fixed] += _cost_epi(fixed, n)
                        up_assign[(st, i)] = fixed
                        continue
                    cand = None
                    for eng in ("dve", "act"):  # pool cannot access PSUM
                        cst = _cost_epi(eng, n)
                        sc = (max(busy[eng] + cst,
                                  *[v for e2, v in busy.items() if e2 != eng]),
                              busy[eng] + cst)
                        if cand is None or sc < cand[0]:
                            cand = (sc, eng, cst)
                    _, eng, cst = cand
                    busy[eng] += cst
                    up_assign[(st, i)] = eng
            mode = modes[G]
            ks = list(range(4 * G, 4 * G + 4))
            rows = ks[::-1] if mode == "fp8" else ks
            for k in rows:
                jb = jbx(k)
                if mode == "fp8":
                    pe += 0.207 * jb + 16.0
                else:
                    pe += 2 * (0.415 * jb + 4.0)
                for c in range(2):
                    cand = None
                    for eng in ("dve", "act"):  # pool: HW gpsimd too slow
                        cst = _cost_egen(eng, mode, jb)
                        sc = (max(busy[eng] + cst,
                                  *[v for e2, v in busy.items() if e2 != eng]),
                              busy[eng] + cst)
                        if cand is None or sc < cand[0]:
                            cand = (sc, eng, cst)
                    _, eng, cst = cand
                    busy[eng] += cst
                    assign[(G, k, c)] = eng
            seen.add(G)
            if (G ^ 1) in seen:
                busy["act"] += _sig_cost(min(N, 32 * (G // 2 + 1)))
        mk = max(max(busy.values()), pe)
        return mk, assign, up_assign, busy, pe

    best = None
    for n8 in range(0, 33):
        modes = ["bf16"] * 32
        for i in range(n8):
            g = min(31, int((i + 0.5) * 32 / n8))
            while modes[g] == "fp8":
                g = (g + 1) % 32
            modes[g] = "fp8"
        for up_all_act in (False, True):
            mk, assign, up_assign, busy, pe = run_walk(modes, up_all_act)
            if best is None or mk < best[0]:
                best = (mk, list(modes), assign, up_assign, busy, pe, n8)
    mk, modes, assign, up_assign, busy, pe, n8 = best
    return {
        "order": order, "triggers": triggers, "modes": modes,
        "assign": assign, "up_assign": up_assign,
        "makespan": mk, "busy": busy, "pe": pe, "n8": n8,
    }


SCHED = _mk_schedule()


# ---------------------------------------------------------------------------
# TileContext variant: split multi-sem-waits (single-wait walrus build)
# ---------------------------------------------------------------------------

class _TC(tile.TileContext):
    MAXW = 1

    def _split_waits_in_list(self, insts):
        out = []
        for inst in insts:
            si = inst.sync_info
            waits = list(si.on_wait) if si is not None else []
            if len(waits) > self.MAXW:
                rest, keep = waits[: -self.MAXW], waits[-self.MAXW :]
                for i in range(0, len(rest), self.MAXW):
                    nop = mybir.InstNoOp(
                        name=self.nc.get_next_instruction_name(),
                        engine=inst.engine,
                        bass_nofuse=True,
                        sync_info=bass_rust.SyncInfo(
                            on_wait=rest[i : i + self.MAXW], on_update=[]
                        ),
                    )
                    out.append(nop)
                inst.sync_info = bass_rust.SyncInfo(
                    on_wait=keep, on_update=list(si.on_update)
                )
            out.append(inst)
        return out

    def _lower_ordered_insts(self, ordered):
        for bb_name in list(ordered.keys()):
            ordered[bb_name] = self._split_waits_in_list(ordered[bb_name])
        return super()._lower_ordered_insts(ordered)

    def _drain_and_barrier(self, tick_clock, wait_clock):
        drain_inst = self.nc.sync.drain()
        wait_clock.add_sem_waits(
            drain_inst.ins, ScopedClock({None: tick_clock.global_clock})
        )
        si = drain_inst.ins.sync_info
        waits = list(si.on_wait) if si is not None else []
        if len(waits) > self.MAXW:
            drain_inst.ins.sync_info = bass_rust.SyncInfo(
                on_wait=waits[: self.MAXW], on_update=list(si.on_update)
            )
            rest = waits[self.MAXW :]
            for i in range(0, len(rest), self.MAXW):
                nop = self.nc.sync.nop(nofuse=True, hint=f"dw{i}")
                nop.ins.sync_info = bass_rust.SyncInfo(
                    on_wait=rest[i : i + self.MAXW], on_update=[]
                )
        self.nc.all_engine_barrier()
        assert self.sems is not None
        popped = self.nc._tile_sem_poison_stack.pop()
        assert popped is self._sem_poison
        self.nc.clear_and_free_semaphores(list(self.sems.allocated().values()))
        self.nc.all_engine_barrier()


def _t2(t, width):
    """View a packed [128, 2*width] tile as [p, t, j]."""
    return t.rearrange("p (t j) -> p t j", t=2)


def _ts(nc, eng):
    """tensor_scalar dispatcher for an engine name."""
    if eng == "dve":
        return nc.vector.tensor_scalar
    if eng == "pool":
        return nc.gpsimd.tensor_scalar
    return None  # act handled via activation


def _egen(nc, engine, out_ap, in_ap, bias_col):
    """e = relu(in + bias_col) with per-partition bias."""
    if engine == "act":
        nc.scalar.activation(out_ap, in_ap, AF.Relu, bias=bias_col)
    else:
        _ts(nc, engine)(
            out=out_ap, in0=in_ap, scalar1=bias_col, scalar2=0.0,
            op0=ALU.add, op1=ALU.max,
        )


def _relu_epi(nc, engine, out_ap, in_ap, bias_col):
    """out = relu(in + bias_col) — upstream h/h1/f1 epilogue."""
    _egen(nc, engine, out_ap, in_ap, bias_col)


class _Ctx:
    pass


# ---------------------------------------------------------------------------
# upstream stages
# ---------------------------------------------------------------------------

def _up_engine(cx, st_name, i, prologue):
    if prologue:
        # fixed prologue assignment tuned for the critical path
        table = {
            ("h0", 0): "dve", ("h0", 1): "act",
            ("h1", 0): "dve", ("h1", 1): "dve",
            ("f0", 0): "act", ("f0", 1): "act",
            ("f1", 0): "dve", ("f1", 1): "dve",
            ("c", 0): "dve", ("c", 1): "act",
            ("a", 0): "dve", ("a", 1): "act",
        }
        return table[(st_name, i)]
    return SCHED["up_assign"][(st_name, i)]


def _upstream_stage(nc, cx, st, stage, prologue=False):
    """Emit one upstream stage.  st is the per-rep state dict."""
    if stage == "xt":
        if prologue:
            st["xt"] = cx.xt_pro  # view into blob tile, already DMA'd
        else:
            t = cx.wpool.tile([128, 2 * NT], FP8, name="xtr", tag="xtr")
            nc.sync.dma_start(t[:, :], cx.xtr[:, :])
            st["xt"] = t
        return

    if stage in ("h0", "h1"):
        if "h8" not in st:
            st["h8"] = cx.wpool.tile([128, 2 * NT], FP8, name="h8", tag="h8")
        dst = st["h8"]
        t0, tn = (0, 512) if stage == "h0" else (512, R)
        for oc in range(2):
            ps = cx.nx_bank(prologue)
            nc.tensor.matmul(
                ps[:, 0:tn],
                lhsT=cx.w1m[oc],
                rhs=_t2(st["xt"], NT)[:, :, t0 : t0 + tn],
                start=True, stop=True, perf_mode=DR,
            )
            eng = _up_engine(cx, stage, oc, prologue)
            _relu_epi(nc, eng,
                      dst[:, oc * NT + t0 : oc * NT + t0 + tn],
                      ps[:, 0:tn], cx.b1c[:, oc : oc + 1])
        return

    if stage == "f0":
        # f8 = relu(ps2/16 + 16*b2) = 16*f   (ACT scale)
        if "f8" not in st:
            st["f8"] = cx.wpool.tile([128, 2 * N], FP8, name="f8", tag="f8")
        dst = st["f8"]
        for oc in range(2):
            ps = cx.nx_bank(prologue)
            nc.tensor.matmul(
                ps[:, 0:512],
                lhsT=cx.w2m[oc],
                rhs=_t2(st["h8"], NT)[:, :, 0:512],
                start=True, stop=True, perf_mode=DR,
            )
            nc.scalar.activation(
                dst[:, oc * N : oc * N + 512], ps[:, 0:512],
                AF.Relu, bias=cx.b2c[:, oc : oc + 1], scale=1.0 / 16.0)
        return

    if stage == "f1":
        # own-token f in bf16: fb1 = relu(ps2 + 256*b2) = 256*f
        st["fb1"] = cx.wpool.tile([128, 2 * R], BF16, name="fb1", tag="fb1")
        dst = st["fb1"]
        for oc in range(2):
            ps = cx.nx_bank(prologue)
            nc.tensor.matmul(
                ps[:, 0:R],
                lhsT=cx.w2m[oc],
                rhs=_t2(st["h8"], NT)[:, :, 512 : 512 + R],
                start=True, stop=True, perf_mode=DR,
            )
            eng = _up_engine(cx, "f1", oc, prologue)
            _relu_epi(nc, eng, dst[:, oc * R : oc * R + R],
                      ps[:, 0:R], cx.b2cc[:, oc : oc + 1])
        return

    if stage == "c":
        # ct = ps3 / 16 = 8*(f·Wb)  in bf16; prologue splits columns so
        # the two epilogue halves run on DVE and ACT concurrently
        st["cts"] = []
        halves = ((0, 256), (256, 512)) if prologue else ((0, 512),)
        for oc in range(2):
            ct = cx.wpool.tile([128, N], BF16, name=f"ct_{oc}",
                               tag=f"ct_{oc}")
            for hi, (c0, c1) in enumerate(halves):
                ps = cx.nx_bank(prologue)
                nc.tensor.matmul(
                    ps[:, 0 : c1 - c0],
                    lhsT=cx.wbm[oc],
                    rhs=_t2(st["f8"], N)[:, :, c0:c1],
                    start=True, stop=True, perf_mode=DR,
                )
                eng = _up_engine(cx, "c", hi if prologue else oc, prologue)
                if eng == "act":
                    nc.scalar.activation(ct[:, c0:c1], ps[:, 0 : c1 - c0],
                                         AF.Identity, scale=1.0 / 16.0)
                else:
                    nc.vector.tensor_scalar_mul(ct[:, c0:c1],
                                                ps[:, 0 : c1 - c0],
                                                1.0 / 16.0)
            st["cts"].append(ct)
        return

    if stage == "a":
        # at = ps_a + 8*bep1, with ps_a = 256f · (Wa/32) = 8*(f·Wa)
        st["ats"] = []
        for oc in range(2):
            at = cx.wpool.tile([128, R], F32, name=f"at_{oc}",
                               tag=f"at_{oc}")
            ps = cx.nx_bank(prologue)
            for kc in range(2):
                nc.tensor.matmul(
                    ps[:, 0:R],
                    lhsT=cx.wam[(oc, kc)],
                    rhs=_t2(st["fb1"], R)[:, kc, :],
                    start=(kc == 0), stop=(kc == 1),
                )
            eng = _up_engine(cx, "a", oc, prologue)
            if eng == "act":
                nc.scalar.activation(at[:, :], ps[:, 0:R], AF.Identity,
                                     bias=cx.bac[:, oc : oc + 1])
            elif eng == "pool":
                nc.gpsimd.tensor_scalar_add(at[:, :], ps[:, 0:R],
                                            cx.bac[:, oc : oc + 1])
            else:
                nc.vector.tensor_scalar_add(at[:, :], ps[:, 0:R],
                                            cx.bac[:, oc : oc + 1])
            st["ats"].append(at)
        return

    raise ValueError(stage)


# ---------------------------------------------------------------------------
# pairwise rep body
# ---------------------------------------------------------------------------

def _rep_body(nc, cx, cts, ats, next_st):
    triggers = SCHED["triggers"] if next_st is not None else {}
    order = SCHED["order"]
    seen = set()
    for it, G in enumerate(order):
        if next_st is not None:
            for stg in triggers.get(it, []):
                _upstream_stage(nc, cx, next_st, stg, prologue=False)
        mode = SCHED["modes"][G]
        tb = (G // 2) % 3
        hb = G % 2
        ps = cx.psum[tb]
        ks = list(range(4 * G, 4 * G + 4))
        rows = ks[::-1] if mode == "fp8" else ks
        for k in rows:
            jb = jbx(k)
            u = k % 4
            if mode == "fp8":
                e8 = cx.e8pool.tile([128, 2 * N], FP8, name="e8", tag="e8",
                                    bufs=8)
                for c in range(2):
                    eng = SCHED["assign"][(G, k, c)]
                    _egen(nc, eng, e8[:, c * N : c * N + jb],
                          cts[c][:, 0:jb], ats[c][:, k : k + 1])
                M = 32 * u + 1
                nc.tensor.matmul(
                    ps[0:M, 512 * hb : 512 * hb + jb],
                    lhsT=_t2(cx.wu8[u], 256)[:, :, 0:M],
                    rhs=_t2(e8, N)[:, :, 0:jb],
                    start=(u == 3), stop=(u == 0),
                    perf_mode=DR, skip_group_check=True,
                )
            else:
                for c in range(2):
                    eb = cx.ebpool.tile([128, N], BF16, name=f"eb{c}",
                                        tag=f"eb{c}", bufs=16)
                    eng = SCHED["assign"][(G, k, c)]
                    _egen(nc, eng, eb[:, 0:jb], cts[c][:, 0:jb],
                          ats[c][:, k : k + 1])
                    nc.tensor.matmul(
                        ps[32 * u : 32 * u + 1, 512 * hb : 512 * hb + jb],
                        lhsT=cx.wep2t[:, c : c + 1],
                        rhs=eb[:, 0:jb],
                        start=(c == 0), stop=(c == 1),
                        tile_position=(0, 32 * u),
                    )
        seen.add(G)
        if (G ^ 1) in seen:
            g = G // 2
            jbb = min(N, 32 * (g + 1))
            ptile = cx.psum[g % 3]
            qin = _t2(ptile, N)[:, :, 0:jbb]
            sc = cx.scpool.tile([128, 2 * N], F32, name="sc", tag="sc",
                                bufs=4)
            qout = _t2(sc, N)[:, :, 0:jbb]
            nc.scalar.activation(qout, qin, AF.Sigmoid,
                                 bias=cx.bep2c[:, 0:1], scale=SIG_SCALE)
            r0 = 8 * g
            dst = cx.y[r0 : r0 + 8, 0:jbb].rearrange("(b u) j -> u b j", b=2)
            src = qout[0:128:32, :, :]
            nc.sync.dma_start(dst, src)


# ---------------------------------------------------------------------------
# build
# ---------------------------------------------------------------------------

def build_nc(reps: int = 1) -> bass.Bass:
    nc = bass.Bass("TRN2", target_bir_lowering=False, debug=False)

    cx = _Ctx()
    blob1 = nc.dram_tensor("blob1", [128, B1_LEN], FP8, kind="ExternalInput")
    w2d = nc.dram_tensor("w2m", [128, 512], FP8, kind="ExternalInput")
    wabd = nc.dram_tensor("wab", [128, 1536], FP8, kind="ExternalInput")
    w8d = nc.dram_tensor("w8full", [128, 2048], FP8, kind="ExternalInput")
    cx.xtr = nc.dram_tensor("xtr", [128, 2 * NT], FP8, kind="ExternalInput")
    cx.y = nc.dram_tensor("y", [R, N], F32, kind="ExternalOutput")

    with _TC(nc) as tc:
        with tc.tile_pool(name="const", bufs=1) as cpool, \
             tc.tile_pool(name="work", bufs=2) as wpool, \
             tc.tile_pool(name="ebpool", bufs=16) as ebpool, \
             tc.tile_pool(name="e8pool", bufs=8) as e8pool, \
             tc.tile_pool(name="scpool", bufs=4) as scpool:

            cx.wpool, cx.ebpool, cx.scpool = wpool, ebpool, scpool
            cx.e8pool = e8pool

            # ---- input DMAs, ordered by first use ----
            tb1 = cpool.tile([128, B1_LEN], FP8, name="tb1")
            nc.sync.dma_start(tb1[:, :], blob1[:, :])
            tw2 = cpool.tile([128, 512], FP8, name="tw2")
            nc.scalar.dma_start(tw2[:, :], w2d[:, :])
            twab = cpool.tile([128, 1536], FP8, name="twab")
            nc.scalar.dma_start(twab[:, :], wabd[:, :])
            tw8 = cpool.tile([128, 2048], FP8, name="tw8")
            nc.sync.dma_start(tw8[:, :], w8d[:, :])

            # ---- views ----
            cx.xt_pro = tb1[:, B1_XT : B1_XT + 1280]
            w1v = tb1[:, B1_W1 : B1_W1 + 512].rearrange(
                "p (t m) -> p t m", t=2)
            cx.w1m = [w1v[:, :, oc * 128 : (oc + 1) * 128] for oc in range(2)]
            cs = tb1[:, B1_CS : B1_CS + 40].bitcast(F32)
            cx.b1c = cs[:, CS_B1 : CS_B1 + 2]
            cx.b2c = cs[:, CS_B2 : CS_B2 + 2]
            cx.bac = cs[:, CS_BA : CS_BA + 2]
            cx.bep2c = cs[:, CS_BEP2 : CS_BEP2 + 1]
            cx.b2cc = cpool.tile([128, 2], F32, name="b2cc")
            w2v = tw2.rearrange("p (t m) -> p t m", t=2)
            cx.w2m = [w2v[:, :, oc * 128 : (oc + 1) * 128] for oc in range(2)]
            wbv = twab[:, 0:512].rearrange("p (t m) -> p t m", t=2)
            cx.wbm = [wbv[:, :, oc * 128 : (oc + 1) * 128] for oc in range(2)]
            wav = twab[:, 512:1536].bitcast(BF16)  # [128, 512] bf16
            wav2 = wav.rearrange("p (kc m) -> p kc m", kc=2)
            cx.wam = {}
            for oc in range(2):
                for kc in range(2):
                    cx.wam[(oc, kc)] = wav2[:, kc, oc * 128 : (oc + 1) * 128]
            cx.wep2t = tb1[:, B1_WEP2 : B1_WEP2 + 4].bitcast(BF16)
            cx.wu8 = [tw8[:, 512 * u : 512 * (u + 1)] for u in range(4)]

            # 256*b2 = 16 * b2c, for the f1 bf16 epilogue (off critical path)
            nc.vector.tensor_scalar_mul(cx.b2cc[:, :], cx.b2c[:, :], 16.0)

            # ---- PSUM ----
            ppp = tc.alloc_tile_pool(name="psum_pair", bufs=1, space="PSUM")
            cx.psum = [ppp.tile([128, 2 * N], F32, name=f"pp{q}")
                       for q in range(3)]
            cx.upsum = [ppp.tile([128, N], F32, name=f"up{q}")
                        for q in range(2)]
            cx.up_rr = 0
            # prologue borrows pairwise tile 2's halves as extra banks
            cx.up_banks_pro = [
                cx.upsum[0], cx.upsum[1],
                cx.psum[2][:, 0:512], cx.psum[2][:, 512:1024],
            ]
            cx.up_rr_pro = 0

            def nx_bank(prologue):
                if prologue:
                    b = cx.up_banks_pro[cx.up_rr_pro % 4]
                    cx.up_rr_pro += 1
                else:
                    b = cx.upsum[cx.up_rr % 2]
                    cx.up_rr += 1
                return b
            cx.nx_bank = nx_bank

            # ---- zero pairwise psum tiles 0,1 (tile 2 via prologue reuse
            # stays finite from upstream values) + PE warm-up ----
            zlhs = cpool.tile([128, 128], BF16, name="zlhs")
            zrhs = cpool.tile([128, N], BF16, name="zrhs")
            nc.vector.memset(zlhs[:, :], 0.0)
            nc.vector.memset(zrhs[:, :], 0.0)
            for q in range(3):
                for half in range(2):
                    nc.tensor.matmul(
                        cx.psum[q][:, 512 * half : 512 * (half + 1)],
                        lhsT=zlhs[:, :], rhs=zrhs[:, 0:512],
                        start=True, stop=True)
            # ---- prologue upstream (critical chain first) ----
            st = {}
            _upstream_stage(nc, cx, st, "xt", prologue=True)
            _upstream_stage(nc, cx, st, "h0", prologue=True)
            _upstream_stage(nc, cx, st, "h1", prologue=True)
            _upstream_stage(nc, cx, st, "f0", prologue=True)
            _upstream_stage(nc, cx, st, "f1", prologue=True)
            _upstream_stage(nc, cx, st, "c", prologue=True)
            _upstream_stage(nc, cx, st, "a", prologue=True)

            for r in range(reps):
                nxt = {} if r + 1 < reps else None
                _rep_body(nc, cx, st["cts"], st["ats"], nxt)
                if nxt is not None:
                    st = nxt

            ppp.release()

    return nc


_NC_CACHE = {}


def _get_nc(reps: int = 1):
    if reps not in _NC_CACHE:
        _NC_CACHE[reps] = build_nc(reps)
    return _NC_CACHE[reps]


# ---------------------------------------------------------------------------
# host packing
# ---------------------------------------------------------------------------

def _pack_w(wT, scale):
    """[256, 256] pre-transposed weight -> [128, 512] fp8 DoubleRow tile."""
    fp8 = ml_dtypes.float8_e4m3
    w8 = np.concatenate([wT[0:128, :], wT[128:256, :]], axis=1)
    return np.ascontiguousarray(w8 * scale).astype(fp8)


def make_in_maps(step_sequence, step_mask, W_gc1, b_gc1, W_gc2, b_gc2,
                 W_ep1, b_ep1, w_ep2, b_ep2):
    x = np.ascontiguousarray(np.asarray(step_sequence, dtype=np.float32))
    W_gc1 = np.asarray(W_gc1, np.float32)
    W_gc2 = np.asarray(W_gc2, np.float32)
    W_ep1 = np.asarray(W_ep1, np.float32)
    b_gc1 = np.asarray(b_gc1, np.float32)
    b_gc2 = np.asarray(b_gc2, np.float32)
    b_ep1 = np.asarray(b_ep1, np.float32)
    w_ep2 = np.asarray(w_ep2, np.float32)
    b_ep2v = np.float32(np.asarray(b_ep2))
    _POST["bep2"] = float(b_ep2v)

    bf16 = ml_dtypes.bfloat16
    fp8 = ml_dtypes.float8_e4m3

    w1m = _pack_w(np.ascontiguousarray(W_gc1.T), 16.0)
    w2m = _pack_w(np.ascontiguousarray(W_gc2.T), 16.0)
    wbm = _pack_w(np.ascontiguousarray(W_ep1[:, H:].T), 8.0)
    # wam bf16 [128, kc(2), m(128)]: lhsT for a-mm (oc,kc) is
    # [:, kc, oc*64? ...] -> actually [:, kc, :] sliced by oc columns of m
    WaT = np.ascontiguousarray(W_ep1[:, :H].T) / 32.0  # [256 h, 256 k-out]
    # contraction chunk kc covers h rows kc*128..; output chunk oc covers
    # k-out columns oc*128..; a-mm (oc,kc) lhsT = [128 h, 128 m]
    wam_full = np.zeros((128, 2, 256), bf16)
    for kc in range(2):
        wam_full[:, kc, :] = WaT[kc * 128 : (kc + 1) * 128, :].astype(bf16)

    cs32 = np.zeros((128, 10), np.float32)
    cs32[:, CS_B1 : CS_B1 + 2] = np.ascontiguousarray(
        (16.0 * b_gc1).reshape(2, 128).T)
    cs32[:, CS_B2 : CS_B2 + 2] = np.ascontiguousarray(
        (16.0 * b_gc2).reshape(2, 128).T)
    cs32[:, CS_BA : CS_BA + 2] = np.ascontiguousarray(
        (E_SCALE * b_ep1).reshape(2, 128).T)
    cs32[:, CS_BEP2] = b_ep2v

    wep2m = np.ascontiguousarray(
        (w_ep2 * W2_SCALE).reshape(2, 128).T).astype(bf16)

    w8full = np.zeros((128, 2048), fp8)
    for u in range(4):
        w8full[:, 512 * u + 32 * u] = (w_ep2[:128] * W2_SCALE).astype(fp8)
        w8full[:, 512 * u + 256 + 32 * u] = (
            w_ep2[128:] * W2_SCALE).astype(fp8)

    wab = np.zeros((128, 1536), np.uint8)
    wab[:, 0:512] = wbm.view(np.uint8)
    wab[:, 512:1536] = wam_full.reshape(128, 512).view(np.uint8)
    wab = wab.view(fp8)

    in_maps = []
    for d in range(NCORES):
        b, ph = divmod(d, 4)
        my_i = np.arange(ph, N, 4)
        xT = x[b].T  # [H, N]
        xTmy = np.ascontiguousarray(x[b][my_i].T)  # [H, R]
        x640 = np.concatenate([xT, xTmy], axis=1)  # [256, 640]
        xt8 = np.ascontiguousarray(
            np.concatenate([x640[0:128, :], x640[128:256, :]], axis=1)
        ).astype(fp8)  # [128, 1280]
        blob = np.zeros((128, B1_LEN), np.uint8)
        blob[:, B1_XT : B1_XT + 1280] = xt8.view(np.uint8)
        blob[:, B1_W1 : B1_W1 + 512] = w1m.view(np.uint8)
        blob[:, B1_CS : B1_CS + 40] = cs32.view(np.uint8)
        blob[:, B1_WEP2 : B1_WEP2 + 4] = wep2m.view(np.uint8)
        m = {
            "blob1": blob.view(fp8), "w2m": w2m, "wab": wab,
            "w8full": w8full, "xtr": xt8,
        }
        in_maps.append(m)
    return in_maps


_MASK_CACHE = {}
_POST = {}


def _tril_mask():
    if "m" not in _MASK_CACHE:
        _MASK_CACHE["m"] = np.tril(np.ones((N, N), np.float32), k=-1)
    return _MASK_CACHE["m"]


def gather_output(results):
    out = np.zeros((B, N, N), np.float32)
    for d in range(NCORES):
        b, ph = divmod(d, 4)
        dev = results[d]["y"]  # [R, N] pre-sigmoid dot products
        for g in range(16):
            J = min(N, 32 * (g + 1))
            ks = np.arange(8 * g, 8 * (g + 1))
            out[b, 4 * ks + ph, :J] = dev[8 * g : 8 * (g + 1), :J]
    out *= _tril_mask()[None, :, :]
    return out


def kernel(**inputs) -> np.ndarray:
    nc = _get_nc()
    in_maps = make_in_maps(**inputs)
    res = run_bass_kernel_spmd(nc, in_maps, core_ids=list(range(NCORES)))
    return gather_output(res.results)
